# revision 17
# baseline (speedup 1.0000x reference)
"""DirectionalGINConv (eps=0) Trainium2 kernel, 8-core SPMD.

  agg_i = sum_{j->i} x_j ; out = relu(relu((x + agg) @ W.T + b))

Strategy (all hardcoded for N=50000, E=800000, C=64, 8 cores):
  - Destination-node sharding: core c owns dst rows [c*6250, (c+1)*6250).
  - Host routes edges: per (core, dst-block-of-128, half) lists, padded to
    K*128 edges. "half" splits edges by src range so gather indices fit in
    int16 (dma_gather limit): half0 uses table base row 0 (src <= 32767),
    half1 uses base row 17232 (src >= 17232); srcs in the overlap are
    assigned to balance the halves.
  - Device per core: dma_gather x rows (fp16, padded to 128ch = 256B elems)
    in block-grouped order; per 128-edge tile build a one-hot selection
    matrix S[e, slot] = (slot[e] == iota) on DVE; segment-sum via PE:
    psum[ch, slot] += G_tile[:, :64].T @ S (PSUM-accumulated over a block's
    tiles); h = psum + x_shard.T; MLP = W.T-stationary matmul; relu+bias on
    ACT; PE transpose back to node-major; DMA out.
"""

import numpy as np
from contextlib import ExitStack

import ml_dtypes

N_NODES = 50000
IN_CH = 64
OUT_CH = 64
N_CORES = 8
SHARD = N_NODES // N_CORES          # 6250
P = 128
NBLK = (SHARD + P - 1) // P         # 49 blocks (last has 106 slots)
BASE1 = 17232                       # half1 table base (50000 - 32768)
CHUNKS = [5]*9 + [2, 2]             # blocks per gather chunk (sum=49)


def _route(src, dst):
    """Vectorized edge routing.

    Returns (idx arrays [2][N_CORES, L] int16, slot arrays [2][N_CORES, L]
    float32 with -1 padding, K) where L = NBLK*K*128.
    """
    src = np.asarray(src, np.int64)
    dst = np.asarray(dst, np.int64)
    core = dst // SHARD
    dloc = dst - core * SHARD
    blk = dloc // P
    slot = dloc - blk * P
    gid = core * NBLK + blk
    ngrp = N_CORES * NBLK
    # categories: 0 = lo-only (half0), 1 = flexible, 2 = hi-only (half1)
    cat = np.where(src < BASE1, 0, np.where(src < 32768, 1, 2)).astype(np.int64)

    n = np.bincount(gid, minlength=ngrp)
    n_lo = np.bincount(gid[cat == 0], minlength=ngrp)
    n_flex = np.bincount(gid[cat == 1], minlength=ngrp)
    half_target = (n + 1) // 2
    f0 = np.clip(half_target - n_lo, 0, n_flex)  # flex edges sent to half0

    # rank within (gid, cat), ordered by src for gather locality
    key_gc = gid * 3 + cat
    order1 = np.lexsort((src, key_gc))
    sk = key_gc[order1]
    starts = np.r_[0, np.flatnonzero(sk[1:] != sk[:-1]) + 1]
    start_of = np.zeros(ngrp * 3, np.int64)
    start_of[sk[starts]] = starts
    rank_gc = np.empty_like(order1)
    rank_gc[order1] = np.arange(len(order1)) - start_of[key_gc][order1]

    half = np.where(cat == 0, 0, np.where(cat == 2, 1, (rank_gc >= f0[gid]).astype(np.int64)))

    # rank within (gid, half), ordered by src
    key_gh = gid * 2 + half
    order2 = np.lexsort((src, key_gh))
    sk2 = key_gh[order2]
    starts2 = np.r_[0, np.flatnonzero(sk2[1:] != sk2[:-1]) + 1]
    start_of2 = np.zeros(ngrp * 2, np.int64)
    start_of2[sk2[starts2]] = starts2
    rank = np.empty_like(order2)
    rank[order2] = np.arange(len(order2)) - start_of2[key_gh][order2]

    cnt_gh = np.bincount(key_gh, minlength=ngrp * 2)
    K = max(1, int(-(-cnt_gh.max() // P)))
    L = NBLK * K * P

    idx_out = [np.zeros((N_CORES, L), np.int16) for _ in range(2)]
    slot_out = [np.full((N_CORES, L), -1.0, np.float32) for _ in range(2)]
    pos = blk * (K * P) + rank
    for h in (0, 1):
        m = half == h
        idx_out[h][core[m], pos[m]] = (src[m] - h * BASE1).astype(np.int16)
        slot_out[h][core[m], pos[m]] = slot[m].astype(np.float32)
    return idx_out, slot_out, K


def _wrap_idx(idx):
    """[L] int16 -> [128, L/16] wrapped (i -> [i%16, i//16]) + replicated."""
    w = idx.reshape(-1, 16).T
    return np.ascontiguousarray(np.tile(w, (8, 1)))


def _slot_tiles(slots, f16):
    """[L] -> [128, L/128] (col t = edges t*128..t*128+127), cast to f16."""
    return np.ascontiguousarray(slots.reshape(-1, P).T).astype(f16)


def _build_program(K):
    import concourse.bacc as bacc
    import concourse.tile as tile
    import concourse.mybir as mybir
    from concourse import library_config

    f16 = mybir.dt.float16
    f32 = mybir.dt.float32
    i16 = mybir.dt.int16

    T_half = NBLK * K
    L = T_half * P
    assert sum(CHUNKS) == NBLK
    CBMAX = max(CHUNKS)

    nc = bacc.Bacc("TRN2", target_bir_lowering=False, debug=False,
                   num_devices=N_CORES, num_swdge_queues=4)
    xg_d = nc.dram_tensor("xg", [N_NODES, 128], f16, kind="ExternalInput")
    i0_d = nc.dram_tensor("i0", [128, L // 16], i16, kind="ExternalInput")
    i1_d = nc.dram_tensor("i1", [128, L // 16], i16, kind="ExternalInput")
    s0_d = nc.dram_tensor("s0", [P, T_half], f16, kind="ExternalInput")
    s1_d = nc.dram_tensor("s1", [P, T_half], f16, kind="ExternalInput")
    xt_d = nc.dram_tensor("xt", [IN_CH, NBLK * P], f32, kind="ExternalInput")
    wt_d = nc.dram_tensor("wt", [IN_CH, OUT_CH], f32, kind="ExternalInput")
    b_d = nc.dram_tensor("b", [OUT_CH, 1], f32, kind="ExternalInput")
    iota_d = nc.dram_tensor("iota", [P, P], f16, kind="ExternalInput")
    ident_d = nc.dram_tensor("ident", [OUT_CH, OUT_CH], f32, kind="ExternalInput")
    out_d = nc.dram_tensor("out", [SHARD, OUT_CH], f32, kind="ExternalOutput")

    with tile.TileContext(nc) as tc, ExitStack() as ctx:
        const_p = ctx.enter_context(tc.tile_pool(name="const", bufs=1))
        gat_p = ctx.enter_context(tc.tile_pool(name="gat", bufs=3))
        sel_p = ctx.enter_context(tc.tile_pool(name="sel", bufs=4))
        h_p = ctx.enter_context(tc.tile_pool(name="h", bufs=3))
        o_p = ctx.enter_context(tc.tile_pool(name="o", bufs=3))
        psum_agg = ctx.enter_context(tc.tile_pool(name="pagg", bufs=3, space="PSUM"))
        psum_mlp = ctx.enter_context(tc.tile_pool(name="pmlp", bufs=2, space="PSUM"))
        psum_tr = ctx.enter_context(tc.tile_pool(name="ptr", bufs=2, space="PSUM"))

        nc.gpsimd.load_library(library_config.mlp)

        i0_t = const_p.tile([128, L // 16], i16)
        i1_t = const_p.tile([128, L // 16], i16)
        s0_t = const_p.tile([P, T_half], f16)
        s1_t = const_p.tile([P, T_half], f16)
        xt_t = const_p.tile([IN_CH, NBLK * P], f32)
        wt_t = const_p.tile([IN_CH, OUT_CH], f32)
        b_t = const_p.tile([OUT_CH, 1], f32)
        iota_t = const_p.tile([P, P], f16)
        ident_t = const_p.tile([OUT_CH, OUT_CH], f32)
        idx_dram = [i0_d, i1_d]
        for t, d in [(s0_t, s0_d), (s1_t, s1_d), (iota_t, iota_d),
                     (xt_t, xt_d), (wt_t, wt_d), (b_t, b_d),
                     (ident_t, ident_d)]:
            nc.scalar.dma_start(out=t[:], in_=d.ap()[:])

        tables = [xg_d.ap()[:, :], xg_d.ap()[BASE1:, :]]
        idx_tiles = [i0_t, i1_t]
        slot_tiles = [s0_t, s1_t]

        qn = 0
        blk0 = 0
        for c, CB in enumerate(CHUNKS):
            g = []
            for h in (0, 1):
                cA = blk0 * K * 8
                cB = (blk0 + CB) * K * 8
                nc.sync.dma_start(out=idx_tiles[h][:, cA:cB],
                                  in_=idx_dram[h].ap()[:, cA:cB])
            for h in (0, 1):
                gt = gat_p.tile([P, CBMAX * K, 128], f16, tag=f"g{h}",
                                name=f"g{h}")
                # split each half-chunk gather across SWDGE queues
                nsp = 2
                base_t = CB * K // nsp
                t0 = 0
                for part in range(nsp):
                    tt = base_t if part < nsp - 1 else CB * K - base_t * (nsp - 1)
                    if tt <= 0:
                        continue
                    n_part = tt * P
                    col0 = (blk0 * K + t0) * 8
                    idx_slice = idx_tiles[h][:, col0: col0 + n_part // 16]
                    nc.gpsimd.dma_gather(gt[:, t0:t0 + tt, :], tables[h],
                                         idx_slice, n_part, n_part, 128,
                                         single_packet=False, queue_num=qn % 4)
                    qn += 1
                    t0 += tt
                g.append(gt)
            for bl in range(CB):
                blk = blk0 + bl
                pa = psum_agg.tile([IN_CH, P], f32, space="PSUM")
                n_mm = 2 * K
                mm = 0
                S_blk = [None, None]
                for h in (0, 1):
                    S_blk[h] = sel_p.tile([P, K, P], f16, name=f"S{h}", tag=f"S{h}")
                    t_idx = blk * K
                    nc.vector.tensor_tensor(
                        out=S_blk[h][:],
                        in0=slot_tiles[h][:, t_idx:t_idx + K].to_broadcast([P, K, P]),
                        in1=iota_t[:][:, None, :].to_broadcast([P, K, P]),
                        op=mybir.AluOpType.is_equal,
                    )
                for h in (0, 1):
                    for k in range(K):
                        nc.tensor.matmul(
                            out=pa[:],
                            lhsT=g[h][:, bl * K + k, :IN_CH],
                            rhs=S_blk[h][:, k, :],
                            start=(mm == 0),
                            stop=(mm == n_mm - 1),
                        )
                        mm += 1
                h_t = h_p.tile([IN_CH, P], f32)
                nc.vector.tensor_add(out=h_t[:], in0=pa[:],
                                     in1=xt_t[:, blk * P:(blk + 1) * P])
                pm = psum_mlp.tile([OUT_CH, P], f32, space="PSUM")
                nc.tensor.matmul(out=pm[:], lhsT=wt_t[:], rhs=h_t[:],
                                 start=True, stop=True)
                r_t = h_p.tile([OUT_CH, P], f32, tag="r")
                nc.scalar.activation(out=r_t[:], in_=pm[:],
                                     func=mybir.ActivationFunctionType.Relu,
                                     bias=b_t[:])
                pt = psum_tr.tile([P, OUT_CH], f32, space="PSUM")
                nc.tensor.transpose(out=pt[:], in_=r_t[:], identity=ident_t[:])
                rows = min(P, SHARD - blk * P)
                o_t = o_p.tile([P, OUT_CH], f32)
                nc.vector.tensor_copy(out=o_t[:], in_=pt[:])
                nc.sync.dma_start(out=out_d.ap()[blk * P: blk * P + rows, :],
                                  in_=o_t[:rows, :])
            blk0 += CB

    nc.compile()
    return nc


def _prepare(x, edge_index, W, b):
    """Host-side routing + per-core input maps. Returns (in_maps, K)."""
    f16np = np.float16
    x = np.asarray(x, np.float32)
    W = np.asarray(W, np.float32)
    b = np.asarray(b, np.float32)
    src = np.asarray(edge_index[0])
    dst = np.asarray(edge_index[1])

    idx_arrs, slot_arrs, K = _route(src, dst)

    xg = np.zeros((N_NODES, 128), f16np)
    xg[:, :IN_CH] = x.astype(f16np)
    iota = np.tile(np.arange(P, dtype=np.float32), (P, 1)).astype(f16np)
    ident = np.eye(OUT_CH, dtype=np.float32)
    wt = np.ascontiguousarray(W.T)
    b2 = np.ascontiguousarray(b.reshape(-1, 1))

    in_maps = []
    for c in range(N_CORES):
        xt = np.zeros((IN_CH, NBLK * P), np.float32)
        xt[:, :SHARD] = x[c * SHARD:(c + 1) * SHARD].T
        in_maps.append({
            "xg": xg,
            "i0": _wrap_idx(idx_arrs[0][c]),
            "i1": _wrap_idx(idx_arrs[1][c]),
            "s0": _slot_tiles(slot_arrs[0][c], f16np),
            "s1": _slot_tiles(slot_arrs[1][c], f16np),
            "xt": np.ascontiguousarray(xt),
            "wt": wt,
            "b": b2,
            "iota": iota,
            "ident": ident,
        })
    return in_maps, K


_CACHE = {}


def _get_program(K):
    if K not in _CACHE:
        _CACHE[K] = _build_program(K)
    return _CACHE[K]


def _best_effort_device_reset():
    """If a previous process wedged the NeuronCores, a reset lets this
    process's run succeed. Harmless (rc=0, state-free) on a healthy device."""
    try:
        import ctypes, jax
        jax.devices()
        lib = ctypes.CDLL("/opt/axon/libaxon_pjrt.so")
        lib.axon_reset.restype = ctypes.c_int64
        lib.axon_reset()
    except Exception:
        pass


def run(x, edge_index, W, b, trace=False):
    from concourse.bass_utils import run_bass_kernel_spmd
    _best_effort_device_reset()
    in_maps, K = _prepare(x, edge_index, W, b)
    nc = _get_program(K)
    res = run_bass_kernel_spmd(nc, in_maps, core_ids=list(range(N_CORES)),
                               trace=trace)
    out = np.concatenate([res.results[c]["out"] for c in range(N_CORES)], axis=0)
    return out.astype(np.float32), res


def kernel(x, edge_index, W, b):
    out, _ = run(x, edge_index, W, b, trace=False)
    return out
